# revision 5
# baseline (speedup 1.0000x reference)
"""Attention pooling (segment softmax + weighted segment-mean) on 8 Trainium2 cores.

Reference computation (per full input):
    logits = leaky_relu(feature @ a, 0.2)                    # [N]
    att    = segment_softmax(logits, batch)                  # [N]
    out    = segment_sum(att[:, None] * feature) / counts    # [1024, 256]

Strategy: batch ids are sorted, so split the 1024 segments into 8 blocks of
128 contiguous segments; each core processes the nodes of its block (padded
to a fixed 25600 nodes). Per 128-node tile the core:
  - computes z = F @ a via a fused DVE multiply+row-reduce,
  - computes ex = exp(leaky_relu(z) - 4) on ACT (max-of-two-exps identity),
  - builds W[p, s] = ex[p] * (seg_local[p] == s) with one DVE tensor_scalar
    (iota is_equal seg, then mult ex),
  - accumulates PSUM segment sums  W.T @ F    [128 segs, 256]
    and segment denominators       W.T @ 1    [128 segs, 1] on TensorE.
The global softmax-max subtraction is replaced by a constant shift (-4):
sums and denoms scale identically so the final ratio is unchanged (logits
are in [-10, 10] for this distribution, so exp stays in fp32 range).
Counts and the final (sums / denom / counts) normalization are O(segments)
and done on host.
"""

from contextlib import ExitStack

import numpy as np

import concourse.bacc as bacc
import concourse.bass as bass
import concourse.tile as tile
from concourse import mybir
from concourse.bass_utils import run_bass_kernel_spmd

N_CORES = 8
P = 128                 # partitions / nodes per tile
H = 256                 # hidden
NSEG = 1024
SEG_PER_CORE = NSEG // N_CORES   # 128
NP = 25600              # padded nodes per core (max real count is ~25.3k)
NT = NP // P            # 200 tiles
EXP_SHIFT = -4.0
NEG_SLOPE = 0.2

_FEAT, _SEGREL, _AREP, _IOTA, _OUT = "feat", "segrel", "arep", "iota", "out"


def _build_program():
    nc = bacc.Bacc("TRN2", target_bir_lowering=False, debug=False)
    feat_d = nc.dram_tensor(_FEAT, [NP, H], mybir.dt.float32, kind="ExternalInput").ap()
    segrel_d = nc.dram_tensor(_SEGREL, [P, NT], mybir.dt.float32, kind="ExternalInput").ap()
    arep_d = nc.dram_tensor(_AREP, [P, H], mybir.dt.float32, kind="ExternalInput").ap()
    iota_d = nc.dram_tensor(_IOTA, [P, P], mybir.dt.float32, kind="ExternalInput").ap()
    out_d = nc.dram_tensor(_OUT, [P, H + 1], mybir.dt.float32, kind="ExternalOutput").ap()

    with tile.TileContext(nc) as tc, ExitStack() as ctx:
        consts = ctx.enter_context(tc.tile_pool(name="consts", bufs=1))
        fpool = ctx.enter_context(tc.tile_pool(name="f", bufs=8))
        ppool = ctx.enter_context(tc.tile_pool(name="prod", bufs=3))
        zpool = ctx.enter_context(tc.tile_pool(name="z", bufs=6))
        wpool = ctx.enter_context(tc.tile_pool(name="w", bufs=4))
        opool = ctx.enter_context(tc.tile_pool(name="o", bufs=1))
        psum = ctx.enter_context(tc.tile_pool(name="psum", bufs=1, space="PSUM"))

        arep_sb = consts.tile([P, H], mybir.dt.float32)
        iota_sb = consts.tile([P, P], mybir.dt.float32)
        segrel_sb = consts.tile([P, NT], mybir.dt.float32)
        ones_sb = consts.tile([P, 1], mybir.dt.float32)
        shift_sb = consts.tile([P, 1], mybir.dt.float32)
        nc.sync.dma_start(arep_sb, arep_d)
        nc.sync.dma_start(iota_sb, iota_d)
        nc.sync.dma_start(segrel_sb, segrel_d)
        nc.vector.memset(ones_sb, 1.0)
        nc.vector.memset(shift_sb, EXP_SHIFT)

        acc = psum.tile([P, H], mybir.dt.float32, tag="acc")
        acc2 = psum.tile([P, 1], mybir.dt.float32, tag="acc2")

        for t in range(NT):
            F = fpool.tile([P, H], mybir.dt.float32)
            nc.sync.dma_start(F, feat_d[t * P:(t + 1) * P, :])

            prod = ppool.tile([P, H], mybir.dt.float32)
            z = zpool.tile([P, 1], mybir.dt.float32, tag="z")
            nc.vector.tensor_tensor(out=prod, in0=F, in1=arep_sb,
                                    op=mybir.AluOpType.mult)
            nc.vector.tensor_reduce(out=z, in_=prod, axis=mybir.AxisListType.X,
                                    op=mybir.AluOpType.add)
            # ex = exp(leaky_relu(z) + EXP_SHIFT) = max(exp(z + s), exp(0.2 z + s))
            e1 = zpool.tile([P, 1], mybir.dt.float32, tag="e1")
            nc.scalar.activation(e1, z, mybir.ActivationFunctionType.Exp,
                                 bias=shift_sb[:, :], scale=1.0)
            e2 = zpool.tile([P, 1], mybir.dt.float32, tag="e2")
            nc.scalar.activation(e2, z, mybir.ActivationFunctionType.Exp,
                                 bias=shift_sb[:, :], scale=NEG_SLOPE)
            ex = zpool.tile([P, 1], mybir.dt.float32, tag="ex")
            nc.vector.tensor_tensor(out=ex, in0=e1, in1=e2, op=mybir.AluOpType.max)

            W = wpool.tile([P, P], mybir.dt.float32)
            nc.vector.tensor_scalar(
                out=W, in0=iota_sb,
                scalar1=segrel_sb[:, t:t + 1], scalar2=ex,
                op0=mybir.AluOpType.is_equal, op1=mybir.AluOpType.mult,
            )
            nc.tensor.matmul(acc, lhsT=W, rhs=F, start=(t == 0), stop=(t == NT - 1))
            nc.tensor.matmul(acc2, lhsT=W, rhs=ones_sb, start=(t == 0), stop=(t == NT - 1))

        out_sb = opool.tile([P, H + 1], mybir.dt.float32)
        nc.vector.tensor_copy(out_sb[:, 0:H], acc)
        nc.vector.tensor_copy(out_sb[:, H:H + 1], acc2)
        nc.sync.dma_start(out_d, out_sb)

    nc.compile()
    return nc


def kernel(feature, a, batch, _trace=False):
    feature = np.asarray(feature, dtype=np.float32)
    a = np.asarray(a, dtype=np.float32)
    batch = np.asarray(batch)
    n = feature.shape[0]
    assert feature.shape == (n, H) and batch.shape == (n,)

    bounds = np.searchsorted(batch, np.arange(0, NSEG + 1, SEG_PER_CORE))
    arep = np.ascontiguousarray(np.broadcast_to(a.reshape(-1), (P, H)), dtype=np.float32)
    iota = np.ascontiguousarray(
        np.broadcast_to(np.arange(P, dtype=np.float32), (P, P)))

    in_maps = []
    for c in range(N_CORES):
        s, e = int(bounds[c]), int(bounds[c + 1])
        cnt = e - s
        assert cnt <= NP, f"core {c} shard {cnt} exceeds padded capacity {NP}"
        feat_c = np.zeros((NP, H), dtype=np.float32)
        feat_c[:cnt] = feature[s:e]
        segrel_c = np.full(NP, P, dtype=np.float32)  # pad id 128 never matches iota
        segrel_c[:cnt] = batch[s:e].astype(np.float32) - c * SEG_PER_CORE
        segrelT = np.ascontiguousarray(segrel_c.reshape(NT, P).T)  # [128, NT]
        in_maps.append({_FEAT: feat_c, _SEGREL: segrelT, _AREP: arep, _IOTA: iota})

    nc = _build_program()
    res = run_bass_kernel_spmd(nc, in_maps, core_ids=list(range(N_CORES)),
                               trace=_trace)

    counts = np.bincount(batch.astype(np.int64), minlength=NSEG).astype(np.float32)
    counts = np.maximum(counts, 1.0)
    out = np.zeros((NSEG, H), dtype=np.float32)
    for c in range(N_CORES):
        blk = res.results[c][_OUT]          # [128, 257]
        sums, denom = blk[:, :H], blk[:, H]
        seg0 = c * SEG_PER_CORE
        safe = np.maximum(denom, 1e-30)[:, None]
        out[seg0:seg0 + SEG_PER_CORE] = np.where(
            denom[:, None] > 0.0,
            sums / safe / counts[seg0:seg0 + SEG_PER_CORE, None],
            0.0,
        )
    if _trace:
        kernel.last_results = res
    return out
